# revision 35
# baseline (speedup 1.0000x reference)
"""Trainium2 Bass kernel for nn_EVModel (gnn_message_passing).

Strategy (8 NeuronCores, SPMD, no collectives):
  - Host: shard the 250k argument edges by owning trigger across the 8
    cores (49 blocks of 128 triggers per core).  Per the sharding hint the
    host gathers the per-edge rows ([rel | rtype | ent], 576 cols) into
    dense bf16 slabs, one [128, T_b*576 + T_b] tile per trigger block
    (x-rows + one-hot codes), with edges grouped by in/out side
    (pure-side tiles).  Triggers are packed into blocks by a degree-aware
    greedy so most blocks need only 5 edge-tiles (3 in + 2 out or 2+3);
    the per-block tile schedule is shared by all cores (SPMD).
  - Device, per block (dense HWDGE loads; no indirect DMA):
      * one DMA load of the block slab (sync queue)
      * per 128-edge tile: DVE builds a one-hot [128e,128t] from the
        codes; PE matmuls with the x-chunks as stationary operands
        accumulate A^T (the per-side segment-sum, transposed) directly in
        PSUM: A^T[c] += X_c^T @ OH  -- no transposes needed.
      * 9 accumulating PE matmuls A^T-chunk @ W-chunk -> args [128t,256]
        (the two 64-row tail chunks of the two sides share one matmul via
        an output-partition-offset segment-sum)
      * one store of args (scalar queue, also HWDGE)
  - Host: gather per-trigger args rows from the 8 slabs, concatenate the
    exact f32 trigger-entity rows (pure pass-through data movement).

Math identity: segsum(x*m) @ W_in + segsum(x*(1-m)) @ W_out, evaluated as
(OH_side^T @ X)^T accumulated transposed, then one 1152-deep matmul with
[W_in ; W_out].  bf16 storage/compute, f32 PSUM accumulation.
"""

import os
import sys

for _p in ("/opt/trn_rl_repo", "/root/.axon_site/_ro/trn_rl_repo"):
    if os.path.isdir(_p) and _p not in sys.path:
        sys.path.insert(0, _p)

import numpy as np
import ml_dtypes

BF16 = ml_dtypes.bfloat16

# ---------------------------------------------------------------- constants
N_ENT, N_REL, N_TRIG, N_ARGS = 100000, 250000, 50000, 250000
ENT_DIM, REL_R, RTYPE_DIM, ROLE_DIM, REL_SIZE = 288, 256, 32, 256, 200
ARG_DIM = REL_R + RTYPE_DIM + ENT_DIM          # 576
OUT_W = ENT_DIM + ROLE_DIM                     # 544
N_CORES = 8
P = 128                                        # partitions / trigger block
BLKS = 49                                      # trigger blocks per core
NBINS = N_CORES * BLKS                         # 392 trigger bins
CW = [128, 128, 128, 128, 64]                  # x-col chunk widths (576)
NWCH = 9                                       # W chunks (64-tails merged)


# ---------------------------------------------------------------- device code
def build_body(nc, tc, aps, sched):
    """Per-core Tile program.  aps: x [P, WTOT] bf16 (all block slabs,
    side by side), w [P, 9*256] bf16, iota [P,P] bf16,
    out [BLKS,P,256] bf16.  sched: list of (tin_b, tout_b) per block.

    Block slab layout per partition: [ x rows of tile 0..T_b-1 | codes ].
    Tiles 0..tin-1 hold in-edges, the rest out-edges; code 128 = pad.
    """
    import concourse.mybir as mybir
    f32, bf16 = mybir.dt.float32, mybir.dt.bfloat16
    eq = mybir.AluOpType.is_equal
    X, W, IOTA, OUT = aps["x"], aps["w"], aps["iota"], aps["out"]
    CODES = aps["codes"]
    t_tot = sum(t[0] + t[1] for t in sched)

    with (
        tc.tile_pool(name="const", bufs=1) as cpool,
        tc.tile_pool(name="xp", bufs=6) as xpool,
        tc.tile_pool(name="ohp", bufs=4) as ohpool,
        tc.tile_pool(name="atp", bufs=2) as atpool,
        tc.tile_pool(name="ap", bufs=3) as apool,
        tc.tile_pool(name="psa", bufs=2, space="PSUM") as psa,
        tc.tile_pool(name="psg", bufs=2, space="PSUM") as psg,
    ):
        # prologue order matters: iota+codes feed the first one-hot, the
        # first x slab feeds the first matmul; wsb is only needed at the
        # first drain stage so it loads after block 0's slab
        iota_sb = cpool.tile([P, P], bf16, name="iota_sb")
        nc.sync.dma_start(out=iota_sb[:], in_=IOTA[:])
        codes_sb = cpool.tile([P, t_tot], f32, name="codes_sb")
        nc.sync.dma_start(out=codes_sb[:], in_=CODES[:])
        wsb = cpool.tile([P, NWCH * ROLE_DIM], bf16, name="wsb")

        def drain_w(b, atps):
            """Copies + W-stage for block b, emitted AFTER block b+1's
            segment-sum: the copies overlap the next segsum on PE and the
            ACT queue never parks on an unresolved wait."""
            # three wide PSUM->SBUF copies (fixed overhead dominates narrow
            # copies); atsb chunk order [in0-3 | out0-3 | merged tails]
            atsb = atpool.tile([P, NWCH * P], bf16, tag="atsb", name="atsb")
            nc.vector.tensor_copy(out=atsb[:, 0:4 * P], in_=atps[0][:, 0:4 * P])
            nc.scalar.copy(out=atsb[:, 4 * P:8 * P], in_=atps[1][:, 0:4 * P])
            nc.scalar.copy(out=atsb[:, 8 * P:9 * P], in_=atps[2][:, 0:P])
            # args = A_cat @ [W_in ; W_out]
            pargs = psg.tile([P, ROLE_DIM], f32, tag="pargs", name="pargs")
            for i in range(NWCH):
                nc.tensor.matmul(
                    out=pargs[:],
                    lhsT=atsb[:, i * P:(i + 1) * P],
                    rhs=wsb[:, i * ROLE_DIM:(i + 1) * ROLE_DIM],
                    start=(i == 0), stop=(i == NWCH - 1))
            return pargs

        def drain_store(b, pargs):
            """Args PSUM->SBUF + store for block b, emitted one block later
            still, so its wait (on the W matmuls) is already resolved when
            the ACT queue reaches it."""
            args_sb = apool.tile([P, ROLE_DIM], bf16, tag="args",
                                 name="args_sb")
            nc.scalar.copy(out=args_sb[:], in_=pargs[:])
            # store via SWDGE: the Pool engine is otherwise idle, so the
            # store's semaphore wait doesn't stall any compute queue
            nc.gpsimd.dma_start(out=OUT[b], in_=args_sb[:])

        off = 0
        ct = 0
        pending = None
        pending2 = None
        for b, (tin, tout) in enumerate(sched):
            T = tin + tout
            xw = T * ARG_DIM
            xall = xpool.tile([P, xw], bf16, tag="x")
            nc.sync.dma_start(out=xall[:], in_=X[:, off:off + xw])
            if b == 0:
                nc.sync.dma_start(out=wsb[:], in_=W[:])

            # ---- segment-sum, transposed, A^T chunks packed into 3 PSUM
            # bank tiles.  HW: start=True marks pending-zero for the WHOLE
            # bank but only on the partitions written, so only the first
            # matmul touching a (bank, partition-range) uses start=True,
            # and all tenants of a bank cover the same partitions.  The two
            # 64-wide tail chunks share bank 2: in-tail on partitions 0:64,
            # out-tail on partitions 64:128 (via output partition offset).
            atps = [psa.tile([P, 4 * P], f32, tag=f"at{j}", name=f"at{j}")
                    for j in range(3)]

            def at_chunk(i, atps=atps):
                # i = side*5 + c -> (bank, region, partition range); the
                # two 64-wide tails pack into bank 2 cols 0:128 on disjoint
                # partition halves
                side, c = divmod(i, 5)
                if c < 4:
                    return atps[side][0:128, c * P:(c + 1) * P]
                if side == 0:
                    return atps[2][0:64, 0:P]
                return atps[2][64:128, 0:P]

            for k in range(T):
                oh = ohpool.tile([P, P], bf16, tag="oh")
                nc.vector.tensor_scalar(
                    out=oh[:], in0=iota_sb[:],
                    scalar1=codes_sb[:, ct + k:ct + k + 1],
                    scalar2=None, op0=eq)
                side = 0 if k < tin else 1
                first = (k == 0 or k == tin)
                for c, cw in enumerate(CW):
                    i = side * 5 + c
                    x0 = k * ARG_DIM + c * P
                    nc.tensor.matmul(out=at_chunk(i),
                                     lhsT=xall[:, x0:x0 + cw],
                                     rhs=oh[:],
                                     start=(first and c in (0, 4)),
                                     stop=(k == tin - 1 or k == T - 1),
                                     skip_group_check=True)

            if pending is not None:
                pb, patps = pending
                drain_store(pb, drain_w(pb, patps))
            pending = (b, atps)
            off += xw
            ct += T
        pb, patps = pending
        drain_store(pb, drain_w(pb, patps))


def build_program(sched):
    import concourse.bacc as bacc
    import concourse.mybir as mybir
    import concourse.tile as tile

    wtot = sum((tin + tout) * ARG_DIM for tin, tout in sched)
    t_tot = sum(tin + tout for tin, tout in sched)
    bf16 = mybir.dt.bfloat16
    f32 = mybir.dt.float32
    nc = bacc.Bacc("TRN2", target_bir_lowering=False, debug=False,
                   num_devices=N_CORES)
    aps = {
        "x": nc.dram_tensor("x", [P, wtot], bf16,
                            kind="ExternalInput").ap(),
        "w": nc.dram_tensor("w", [P, NWCH * ROLE_DIM], bf16,
                            kind="ExternalInput").ap(),
        "iota": nc.dram_tensor("iota", [P, P], bf16,
                               kind="ExternalInput").ap(),
        "codes": nc.dram_tensor("codes", [P, t_tot], f32,
                                kind="ExternalInput").ap(),
        "out": nc.dram_tensor("out", [BLKS, P, ROLE_DIM], bf16,
                              kind="ExternalOutput").ap(),
    }
    with tile.TileContext(nc) as tc:
        build_body(nc, tc, aps, sched)
    nc.compile()
    return nc


# ---------------------------------------------------------------- host prep
def pack_bins(din, dout):
    """Degree-aware greedy: pack triggers into 392 bins of 128 so each
    bin's per-side edge sums land just under multiples of 128.  Returns
    (bin_of_trigger, realized per-bin (in,out) sums)."""
    deg = np.stack([din, np.asarray(dout)], axis=1).astype(np.int64)
    # bucket triggers by (din, dout); virtual (0,0) triggers pad to 50176
    order = np.lexsort((deg[:, 1], deg[:, 0]))
    di, do = deg[order, 0], deg[order, 1]
    # bucket boundaries
    chg = np.nonzero((np.diff(di) != 0) | (np.diff(do) != 0))[0] + 1
    starts = np.concatenate([[0], chg])
    ends = np.concatenate([chg, [N_TRIG]])
    bi = di[starts].copy()
    bo = do[starts].copy()
    cnt = (ends - starts).astype(np.int64)
    pos = starts.copy()          # next unassigned trigger per bucket
    ndummy = NBINS * P - N_TRIG  # virtual zero-degree triggers
    if not (bi[0] == 0 and bo[0] == 0):
        bi = np.concatenate([[0], bi]); bo = np.concatenate([[0], bo])
        cnt = np.concatenate([[0], cnt]); pos = np.concatenate([[0], pos])
        starts = np.concatenate([[0], starts])
        ends = np.concatenate([[0], ends])
    cnt[0] += ndummy

    rem_in, rem_out = int(din.sum()), int(dout.sum())
    bin_of = np.empty(N_TRIG, np.int64)
    bin_fill = np.zeros(NBINS, np.int64)
    sums = np.zeros((NBINS, 2), np.int64)
    f = bi.astype(np.float64)
    g = bo.astype(np.float64)
    for q in range(NBINS):
        rb = NBINS - q
        # bin type: lean the side with more remaining edges
        ai = rem_in / rb
        ao = rem_out / rb
        ti = 128.0 * np.ceil(max(ai, 1.0) / 128.0)
        to = 128.0 * np.ceil(max(ao, 1.0) / 128.0)
        # prefer a 5-tile bin when the average allows
        if ti + to > 640 and ai + ao <= 636:
            if ai >= ao:
                ti, to = 384.0, 256.0
            else:
                ti, to = 256.0, 384.0
        si = so = 0.0
        for r in range(P, 0, -1):
            ni = (ti - si) / r
            no = (to - so) / r
            ok = cnt > 0
            # never overshoot the caps
            okc = ok & (f <= ti - si) & (g <= to - so)
            if not okc.any():
                okc = ok
            score = (f - ni) ** 2 + (g - no) ** 2
            score[~okc] = np.inf
            j = int(np.argmin(score))
            cnt[j] -= 1
            si += f[j]
            so += g[j]
            sums[q, 0] += bi[j]
            sums[q, 1] += bo[j]
            # assign a real trigger id if the bucket still has one
            # (bucket 0 also holds the virtual padding triggers)
            if pos[j] < ends[j]:
                bin_of[order[pos[j]]] = q
                pos[j] += 1
            bin_fill[q] += 1
        rem_in -= int(sums[q, 0])
        rem_out -= int(sums[q, 1])
    return bin_of, sums


def make_schedule(sums):
    """Group the 392 bins 8-per-block-index by tile class; returns
    (bin -> (core, blk) slot base order, sched list)."""
    ib = np.maximum(1, np.ceil(sums[:, 0] / P).astype(np.int64))
    ob = np.maximum(1, np.ceil(sums[:, 1] / P).astype(np.int64))
    order = np.lexsort((ob, ib))          # group equal classes together
    sched = []
    bin_slot = np.empty(NBINS, np.int64)  # bin -> global block slot
    for b in range(BLKS):
        grp = order[b * N_CORES:(b + 1) * N_CORES]
        sched.append((int(ib[grp].max()), int(ob[grp].max())))
        for c, q in enumerate(grp):
            bin_slot[q] = c * BLKS + b    # core c, block b
    return bin_slot, sched


def host_prep(inputs):
    rtype_ids = np.asarray(inputs["rtype_ids"], np.int64)
    arg_trig = np.asarray(inputs["arg_trig"], np.int64)
    arg_rel = np.asarray(inputs["arg_rel"], np.int64)
    arg_ent = np.asarray(inputs["arg_ent"], np.int64)
    arg_is_in = np.asarray(inputs["arg_is_in"], np.int64)

    rel_bf = np.asarray(inputs["rel_embeds"], np.float32).astype(BF16)
    rtt_bf = np.asarray(inputs["rtype_table"], np.float32).astype(BF16)
    ent_bf = np.asarray(inputs["ent_embeds"], np.float32).astype(BF16)

    din = np.bincount(arg_trig[arg_is_in == 1], minlength=N_TRIG)
    dout = np.bincount(arg_trig[arg_is_in == 0], minlength=N_TRIG)
    bin_of, sums = pack_bins(din, dout)
    bin_slot, sched = make_schedule(sums)

    # trigger -> (global block slot, position within block)
    blk_of_trig = bin_slot[bin_of]
    pos_order = np.lexsort((np.arange(N_TRIG), blk_of_trig))
    pos_of_trig = np.empty(N_TRIG, np.int64)
    pos_in_blk = np.arange(N_TRIG) - np.concatenate(
        [[0], np.cumsum(np.bincount(blk_of_trig, minlength=NBINS))])[
        blk_of_trig[pos_order]]
    pos_of_trig[pos_order] = pos_in_blk
    slot = blk_of_trig * P + pos_of_trig          # trigger -> output slot

    # per-block slab offsets
    tins = np.array([s[0] for s in sched], np.int64)
    touts = np.array([s[1] for s in sched], np.int64)
    ts = tins + touts
    xws = ts * ARG_DIM
    offs = np.concatenate([[0], np.cumsum(xws)])
    wtot = int(offs[-1])
    cts = np.concatenate([[0], np.cumsum(ts)])
    t_tot = int(cts[-1])

    # per-edge placement
    eblk = blk_of_trig[arg_trig]                  # global block slot (0..391)
    b_of = eblk % BLKS                            # block index within core
    core = eblk // BLKS
    epos = pos_of_trig[arg_trig]
    side = 1 - arg_is_in
    key = eblk * 2 + side
    order = np.argsort(key, kind="stable")
    ks = key[order]
    cntk = np.bincount(ks, minlength=2 * NBINS)
    grp_start = np.concatenate([[0], np.cumsum(cntk)])
    rank = np.arange(N_ARGS) - grp_start[ks]
    tin_e = tins[b_of[order]]
    ek = np.where(ks % 2 == 0, rank // P, tin_e + rank // P)
    ep = rank % P
    assert (np.where(ks % 2 == 0, rank, -1) < tin_e * P).all()
    assert (np.where(ks % 2 == 1, rank, -1) < touts[b_of[order]] * P).all()

    e = order
    X = np.zeros((N_CORES, P, wtot), BF16)
    flat = X.reshape(-1)
    base = (core[e] * P + ep) * wtot + offs[b_of[e]] + ek * ARG_DIM
    flat[base[:, None] + np.arange(REL_R)] = rel_bf[arg_rel[e]]
    flat[base[:, None] + (REL_R + np.arange(RTYPE_DIM))] = \
        rtt_bf[rtype_ids[arg_rel[e]]]
    flat[base[:, None] + (REL_R + RTYPE_DIM + np.arange(ENT_DIM))] = \
        ent_bf[arg_ent[e]]
    # codes (pad = 128), separate resident f32 table [cores, P, t_tot]
    codes = np.full((N_CORES, P, t_tot), np.float32(P), np.float32)
    codes.reshape(-1)[(core[e] * P + ep) * t_tot + cts[b_of[e]] + ek] = \
        epos[e].astype(np.float32)

    # W chunks [128, 9, 256] in atsb order [in0-3 | out0-3 | merged tails]
    wstack = np.concatenate([np.asarray(inputs["W_in"], np.float32),
                             np.asarray(inputs["W_out"], np.float32)], axis=0)
    wh = np.zeros((P, NWCH, ROLE_DIM), np.float32)
    for c in range(4):
        wh[:, c, :] = wstack[c * P:(c + 1) * P]
        wh[:, 4 + c, :] = wstack[ARG_DIM + c * P:ARG_DIM + (c + 1) * P]
    wh[0:64, 8, :] = wstack[512:576]
    wh[64:128, 8, :] = wstack[ARG_DIM + 512:ARG_DIM + 576]
    wh = wh.reshape(P, NWCH * ROLE_DIM).astype(BF16)

    iota = np.ascontiguousarray(
        np.broadcast_to(np.arange(P, dtype=np.float32), (P, P))).astype(BF16)

    per_core = [{"x": np.ascontiguousarray(X[c]), "w": wh, "iota": iota,
                 "codes": np.ascontiguousarray(codes[c])}
                for c in range(N_CORES)]
    return per_core, slot, tuple(sched)


_PROGRAM_CACHE = {}


def kernel(**inputs):
    from concourse.bass_utils import run_bass_kernel_spmd

    per_core, slot, sched = host_prep(inputs)
    if sched not in _PROGRAM_CACHE:
        _PROGRAM_CACHE[sched] = build_program(sched)
    nc = _PROGRAM_CACHE[sched]

    res = run_bass_kernel_spmd(nc, per_core, core_ids=list(range(N_CORES)))
    args = np.concatenate(
        [np.asarray(res.results[c]["out"]).reshape(BLKS * P, ROLE_DIM)
         for c in range(N_CORES)], axis=0)[slot].astype(np.float32)
    out = np.empty((N_TRIG, OUT_W), np.float32)
    out[:, 0:ENT_DIM] = np.asarray(inputs["ent_embeds"],
                                   np.float32)[np.asarray(
                                       inputs["trig_ent_id"], np.int64)]
    out[:, ENT_DIM:] = args
    return out


# revision 37
# speedup vs baseline: 1.0023x; 1.0023x over previous
"""Trainium2 Bass kernel for nn_EVModel (gnn_message_passing).

Strategy (8 NeuronCores, SPMD, no collectives):
  - Host: shard the 250k argument edges by owning trigger across the 8
    cores (49 blocks of 128 triggers per core).  Per the sharding hint the
    host gathers the per-edge rows ([rel | rtype | ent], 576 cols) into
    dense bf16 slabs, one [128, T_b*576 + T_b] tile per trigger block
    (x-rows + one-hot codes), with edges grouped by in/out side
    (pure-side tiles).  Triggers are packed into blocks by a degree-aware
    greedy so most blocks need only 5 edge-tiles (3 in + 2 out or 2+3);
    the per-block tile schedule is shared by all cores (SPMD).
  - Device, per block (dense HWDGE loads; no indirect DMA):
      * one DMA load of the block slab (sync queue)
      * per 128-edge tile: DVE builds a one-hot [128e,128t] from the
        codes; PE matmuls with the x-chunks as stationary operands
        accumulate A^T (the per-side segment-sum, transposed) directly in
        PSUM: A^T[c] += X_c^T @ OH  -- no transposes needed.
      * 9 accumulating PE matmuls A^T-chunk @ W-chunk -> args [128t,256]
        (the two 64-row tail chunks of the two sides share one matmul via
        an output-partition-offset segment-sum)
      * one store of args (scalar queue, also HWDGE)
  - Host: gather per-trigger args rows from the 8 slabs, concatenate the
    exact f32 trigger-entity rows (pure pass-through data movement).

Math identity: segsum(x*m) @ W_in + segsum(x*(1-m)) @ W_out, evaluated as
(OH_side^T @ X)^T accumulated transposed, then one 1152-deep matmul with
[W_in ; W_out].  bf16 storage/compute, f32 PSUM accumulation.
"""

import os
import sys

for _p in ("/opt/trn_rl_repo", "/root/.axon_site/_ro/trn_rl_repo"):
    if os.path.isdir(_p) and _p not in sys.path:
        sys.path.insert(0, _p)

import numpy as np
import ml_dtypes

BF16 = ml_dtypes.bfloat16

# ---------------------------------------------------------------- constants
N_ENT, N_REL, N_TRIG, N_ARGS = 100000, 250000, 50000, 250000
ENT_DIM, REL_R, RTYPE_DIM, ROLE_DIM, REL_SIZE = 288, 256, 32, 256, 200
ARG_DIM = REL_R + RTYPE_DIM + ENT_DIM          # 576
OUT_W = ENT_DIM + ROLE_DIM                     # 544
N_CORES = 8
P = 128                                        # partitions / trigger block
BLKS = 49                                      # trigger blocks per core
NBINS = N_CORES * BLKS                         # 392 trigger bins
CW = [128, 128, 128, 128, 64]                  # x-col chunk widths (576)
NWCH = 9                                       # W chunks (64-tails merged)


# ---------------------------------------------------------------- device code
def build_body(nc, tc, aps, sched):
    """Per-core Tile program.  aps: x [P, WTOT] bf16 (all block slabs,
    side by side), w [P, 9*256] bf16, iota [P,P] bf16,
    out [BLKS,P,256] bf16.  sched: list of (tin_b, tout_b) per block.

    Block slab layout per partition: [ x rows of tile 0..T_b-1 | codes ].
    Tiles 0..tin-1 hold in-edges, the rest out-edges; code 128 = pad.
    """
    import concourse.mybir as mybir
    f32, bf16 = mybir.dt.float32, mybir.dt.bfloat16
    eq = mybir.AluOpType.is_equal
    X, W, IOTA, OUT = aps["x"], aps["w"], aps["iota"], aps["out"]
    CODES = aps["codes"]
    t_tot = sum(t[0] + t[1] for t in sched)

    with (
        tc.tile_pool(name="const", bufs=1) as cpool,
        tc.tile_pool(name="xp", bufs=6) as xpool,
        tc.tile_pool(name="ohp", bufs=4) as ohpool,
        tc.tile_pool(name="atp", bufs=2) as atpool,
        tc.tile_pool(name="ap", bufs=3) as apool,
        tc.tile_pool(name="psa", bufs=2, space="PSUM") as psa,
        tc.tile_pool(name="psg", bufs=2, space="PSUM") as psg,
    ):
        wsb = cpool.tile([P, NWCH * ROLE_DIM], bf16, name="wsb")
        nc.sync.dma_start(out=wsb[:], in_=W[:])
        iota_sb = cpool.tile([P, P], bf16, name="iota_sb")
        nc.sync.dma_start(out=iota_sb[:], in_=IOTA[:])
        codes_sb = cpool.tile([P, t_tot], f32, name="codes_sb")
        nc.sync.dma_start(out=codes_sb[:], in_=CODES[:])

        def drain_w(b, atps):
            """Copies + W-stage for block b, emitted AFTER block b+1's
            segment-sum: the copies overlap the next segsum on PE and the
            ACT queue never parks on an unresolved wait."""
            # three wide PSUM->SBUF copies (fixed overhead dominates narrow
            # copies); atsb chunk order [in0-3 | out0-3 | merged tails]
            atsb = atpool.tile([P, NWCH * P], bf16, tag="atsb", name="atsb")
            nc.vector.tensor_copy(out=atsb[:, 0:4 * P], in_=atps[0][:, 0:4 * P])
            nc.scalar.copy(out=atsb[:, 4 * P:8 * P], in_=atps[1][:, 0:4 * P])
            nc.scalar.copy(out=atsb[:, 8 * P:9 * P], in_=atps[2][:, 0:P])
            # args = A_cat @ [W_in ; W_out]
            pargs = psg.tile([P, ROLE_DIM], f32, tag="pargs", name="pargs")
            for i in range(NWCH):
                nc.tensor.matmul(
                    out=pargs[:],
                    lhsT=atsb[:, i * P:(i + 1) * P],
                    rhs=wsb[:, i * ROLE_DIM:(i + 1) * ROLE_DIM],
                    start=(i == 0), stop=(i == NWCH - 1))
            return pargs

        def drain_store(b, pargs):
            """Args PSUM->SBUF + store for block b, emitted one block later
            still, so its wait (on the W matmuls) is already resolved when
            the ACT queue reaches it."""
            args_sb = apool.tile([P, ROLE_DIM], bf16, tag="args",
                                 name="args_sb")
            nc.scalar.copy(out=args_sb[:], in_=pargs[:])
            # store via SWDGE: the Pool engine is otherwise idle, so the
            # store's semaphore wait doesn't stall any compute queue
            nc.gpsimd.dma_start(out=OUT[b], in_=args_sb[:])

        off = 0
        ct = 0
        pending = None
        pending2 = None
        for b, (tin, tout) in enumerate(sched):
            T = tin + tout
            xw = T * ARG_DIM
            xall = xpool.tile([P, xw], bf16, tag="x")
            nc.sync.dma_start(out=xall[:], in_=X[:, off:off + xw])
            if b == 0:
                # PE clock warm-up: ~80 throwaway matmuls overlap the
                # prologue DMA wait so the real stream starts at full clock
                warm = psg.tile([P, ROLE_DIM], f32, tag="pargs", name="warm")
                for _ in range(80):
                    nc.tensor.matmul(out=warm[:, 0:P], lhsT=iota_sb[:],
                                     rhs=iota_sb[:], start=True, stop=True)

            # ---- segment-sum, transposed, A^T chunks packed into 3 PSUM
            # bank tiles.  HW: start=True marks pending-zero for the WHOLE
            # bank but only on the partitions written, so only the first
            # matmul touching a (bank, partition-range) uses start=True,
            # and all tenants of a bank cover the same partitions.  The two
            # 64-wide tail chunks share bank 2: in-tail on partitions 0:64,
            # out-tail on partitions 64:128 (via output partition offset).
            atps = [psa.tile([P, 4 * P], f32, tag=f"at{j}", name=f"at{j}")
                    for j in range(3)]

            def at_chunk(i, atps=atps):
                # i = side*5 + c -> (bank, region, partition range); the
                # two 64-wide tails pack into bank 2 cols 0:128 on disjoint
                # partition halves
                side, c = divmod(i, 5)
                if c < 4:
                    return atps[side][0:128, c * P:(c + 1) * P]
                if side == 0:
                    return atps[2][0:64, 0:P]
                return atps[2][64:128, 0:P]

            for k in range(T):
                oh = ohpool.tile([P, P], bf16, tag="oh")
                nc.vector.tensor_scalar(
                    out=oh[:], in0=iota_sb[:],
                    scalar1=codes_sb[:, ct + k:ct + k + 1],
                    scalar2=None, op0=eq)
                side = 0 if k < tin else 1
                first = (k == 0 or k == tin)
                for c, cw in enumerate(CW):
                    i = side * 5 + c
                    x0 = k * ARG_DIM + c * P
                    nc.tensor.matmul(out=at_chunk(i),
                                     lhsT=xall[:, x0:x0 + cw],
                                     rhs=oh[:],
                                     start=(first and c in (0, 4)),
                                     stop=(k == tin - 1 or k == T - 1),
                                     skip_group_check=True)

            if pending is not None:
                pb, patps = pending
                drain_store(pb, drain_w(pb, patps))
            pending = (b, atps)
            off += xw
            ct += T
        pb, patps = pending
        drain_store(pb, drain_w(pb, patps))


def build_program(sched):
    import concourse.bacc as bacc
    import concourse.mybir as mybir
    import concourse.tile as tile

    wtot = sum((tin + tout) * ARG_DIM for tin, tout in sched)
    t_tot = sum(tin + tout for tin, tout in sched)
    bf16 = mybir.dt.bfloat16
    f32 = mybir.dt.float32
    nc = bacc.Bacc("TRN2", target_bir_lowering=False, debug=False,
                   num_devices=N_CORES)
    aps = {
        "x": nc.dram_tensor("x", [P, wtot], bf16,
                            kind="ExternalInput").ap(),
        "w": nc.dram_tensor("w", [P, NWCH * ROLE_DIM], bf16,
                            kind="ExternalInput").ap(),
        "iota": nc.dram_tensor("iota", [P, P], bf16,
                               kind="ExternalInput").ap(),
        "codes": nc.dram_tensor("codes", [P, t_tot], f32,
                                kind="ExternalInput").ap(),
        "out": nc.dram_tensor("out", [BLKS, P, ROLE_DIM], bf16,
                              kind="ExternalOutput").ap(),
    }
    with tile.TileContext(nc) as tc:
        build_body(nc, tc, aps, sched)
    nc.compile()
    return nc


# ---------------------------------------------------------------- host prep
def pack_bins(din, dout):
    """Degree-aware greedy: pack triggers into 392 bins of 128 so each
    bin's per-side edge sums land just under multiples of 128.  Returns
    (bin_of_trigger, realized per-bin (in,out) sums)."""
    deg = np.stack([din, np.asarray(dout)], axis=1).astype(np.int64)
    # bucket triggers by (din, dout); virtual (0,0) triggers pad to 50176
    order = np.lexsort((deg[:, 1], deg[:, 0]))
    di, do = deg[order, 0], deg[order, 1]
    # bucket boundaries
    chg = np.nonzero((np.diff(di) != 0) | (np.diff(do) != 0))[0] + 1
    starts = np.concatenate([[0], chg])
    ends = np.concatenate([chg, [N_TRIG]])
    bi = di[starts].copy()
    bo = do[starts].copy()
    cnt = (ends - starts).astype(np.int64)
    pos = starts.copy()          # next unassigned trigger per bucket
    ndummy = NBINS * P - N_TRIG  # virtual zero-degree triggers
    if not (bi[0] == 0 and bo[0] == 0):
        bi = np.concatenate([[0], bi]); bo = np.concatenate([[0], bo])
        cnt = np.concatenate([[0], cnt]); pos = np.concatenate([[0], pos])
        starts = np.concatenate([[0], starts])
        ends = np.concatenate([[0], ends])
    cnt[0] += ndummy

    rem_in, rem_out = int(din.sum()), int(dout.sum())
    bin_of = np.empty(N_TRIG, np.int64)
    bin_fill = np.zeros(NBINS, np.int64)
    sums = np.zeros((NBINS, 2), np.int64)
    f = bi.astype(np.float64)
    g = bo.astype(np.float64)
    for q in range(NBINS):
        rb = NBINS - q
        # bin type: lean the side with more remaining edges
        ai = rem_in / rb
        ao = rem_out / rb
        ti = 128.0 * np.ceil(max(ai, 1.0) / 128.0)
        to = 128.0 * np.ceil(max(ao, 1.0) / 128.0)
        # prefer a 5-tile bin when the average allows
        if ti + to > 640 and ai + ao <= 636:
            if ai >= ao:
                ti, to = 384.0, 256.0
            else:
                ti, to = 256.0, 384.0
        si = so = 0.0
        for r in range(P, 0, -1):
            ni = (ti - si) / r
            no = (to - so) / r
            ok = cnt > 0
            # never overshoot the caps
            okc = ok & (f <= ti - si) & (g <= to - so)
            if not okc.any():
                okc = ok
            score = (f - ni) ** 2 + (g - no) ** 2
            score[~okc] = np.inf
            j = int(np.argmin(score))
            cnt[j] -= 1
            si += f[j]
            so += g[j]
            sums[q, 0] += bi[j]
            sums[q, 1] += bo[j]
            # assign a real trigger id if the bucket still has one
            # (bucket 0 also holds the virtual padding triggers)
            if pos[j] < ends[j]:
                bin_of[order[pos[j]]] = q
                pos[j] += 1
            bin_fill[q] += 1
        rem_in -= int(sums[q, 0])
        rem_out -= int(sums[q, 1])
    return bin_of, sums


def make_schedule(sums):
    """Group the 392 bins 8-per-block-index by tile class; returns
    (bin -> (core, blk) slot base order, sched list)."""
    ib = np.maximum(1, np.ceil(sums[:, 0] / P).astype(np.int64))
    ob = np.maximum(1, np.ceil(sums[:, 1] / P).astype(np.int64))
    order = np.lexsort((ob, ib))          # group equal classes together
    sched = []
    bin_slot = np.empty(NBINS, np.int64)  # bin -> global block slot
    for b in range(BLKS):
        grp = order[b * N_CORES:(b + 1) * N_CORES]
        sched.append((int(ib[grp].max()), int(ob[grp].max())))
        for c, q in enumerate(grp):
            bin_slot[q] = c * BLKS + b    # core c, block b
    return bin_slot, sched


def host_prep(inputs):
    rtype_ids = np.asarray(inputs["rtype_ids"], np.int64)
    arg_trig = np.asarray(inputs["arg_trig"], np.int64)
    arg_rel = np.asarray(inputs["arg_rel"], np.int64)
    arg_ent = np.asarray(inputs["arg_ent"], np.int64)
    arg_is_in = np.asarray(inputs["arg_is_in"], np.int64)

    rel_bf = np.asarray(inputs["rel_embeds"], np.float32).astype(BF16)
    rtt_bf = np.asarray(inputs["rtype_table"], np.float32).astype(BF16)
    ent_bf = np.asarray(inputs["ent_embeds"], np.float32).astype(BF16)

    din = np.bincount(arg_trig[arg_is_in == 1], minlength=N_TRIG)
    dout = np.bincount(arg_trig[arg_is_in == 0], minlength=N_TRIG)
    bin_of, sums = pack_bins(din, dout)
    bin_slot, sched = make_schedule(sums)

    # trigger -> (global block slot, position within block)
    blk_of_trig = bin_slot[bin_of]
    pos_order = np.lexsort((np.arange(N_TRIG), blk_of_trig))
    pos_of_trig = np.empty(N_TRIG, np.int64)
    pos_in_blk = np.arange(N_TRIG) - np.concatenate(
        [[0], np.cumsum(np.bincount(blk_of_trig, minlength=NBINS))])[
        blk_of_trig[pos_order]]
    pos_of_trig[pos_order] = pos_in_blk
    slot = blk_of_trig * P + pos_of_trig          # trigger -> output slot

    # per-block slab offsets
    tins = np.array([s[0] for s in sched], np.int64)
    touts = np.array([s[1] for s in sched], np.int64)
    ts = tins + touts
    xws = ts * ARG_DIM
    offs = np.concatenate([[0], np.cumsum(xws)])
    wtot = int(offs[-1])
    cts = np.concatenate([[0], np.cumsum(ts)])
    t_tot = int(cts[-1])

    # per-edge placement
    eblk = blk_of_trig[arg_trig]                  # global block slot (0..391)
    b_of = eblk % BLKS                            # block index within core
    core = eblk // BLKS
    epos = pos_of_trig[arg_trig]
    side = 1 - arg_is_in
    key = eblk * 2 + side
    order = np.argsort(key, kind="stable")
    ks = key[order]
    cntk = np.bincount(ks, minlength=2 * NBINS)
    grp_start = np.concatenate([[0], np.cumsum(cntk)])
    rank = np.arange(N_ARGS) - grp_start[ks]
    tin_e = tins[b_of[order]]
    ek = np.where(ks % 2 == 0, rank // P, tin_e + rank // P)
    ep = rank % P
    assert (np.where(ks % 2 == 0, rank, -1) < tin_e * P).all()
    assert (np.where(ks % 2 == 1, rank, -1) < touts[b_of[order]] * P).all()

    e = order
    X = np.zeros((N_CORES, P, wtot), BF16)
    flat = X.reshape(-1)
    base = (core[e] * P + ep) * wtot + offs[b_of[e]] + ek * ARG_DIM
    flat[base[:, None] + np.arange(REL_R)] = rel_bf[arg_rel[e]]
    flat[base[:, None] + (REL_R + np.arange(RTYPE_DIM))] = \
        rtt_bf[rtype_ids[arg_rel[e]]]
    flat[base[:, None] + (REL_R + RTYPE_DIM + np.arange(ENT_DIM))] = \
        ent_bf[arg_ent[e]]
    # codes (pad = 128), separate resident f32 table [cores, P, t_tot]
    codes = np.full((N_CORES, P, t_tot), np.float32(P), np.float32)
    codes.reshape(-1)[(core[e] * P + ep) * t_tot + cts[b_of[e]] + ek] = \
        epos[e].astype(np.float32)

    # W chunks [128, 9, 256] in atsb order [in0-3 | out0-3 | merged tails]
    wstack = np.concatenate([np.asarray(inputs["W_in"], np.float32),
                             np.asarray(inputs["W_out"], np.float32)], axis=0)
    wh = np.zeros((P, NWCH, ROLE_DIM), np.float32)
    for c in range(4):
        wh[:, c, :] = wstack[c * P:(c + 1) * P]
        wh[:, 4 + c, :] = wstack[ARG_DIM + c * P:ARG_DIM + (c + 1) * P]
    wh[0:64, 8, :] = wstack[512:576]
    wh[64:128, 8, :] = wstack[ARG_DIM + 512:ARG_DIM + 576]
    wh = wh.reshape(P, NWCH * ROLE_DIM).astype(BF16)

    iota = np.ascontiguousarray(
        np.broadcast_to(np.arange(P, dtype=np.float32), (P, P))).astype(BF16)

    per_core = [{"x": np.ascontiguousarray(X[c]), "w": wh, "iota": iota,
                 "codes": np.ascontiguousarray(codes[c])}
                for c in range(N_CORES)]
    return per_core, slot, tuple(sched)


_PROGRAM_CACHE = {}


def kernel(**inputs):
    from concourse.bass_utils import run_bass_kernel_spmd

    per_core, slot, sched = host_prep(inputs)
    if sched not in _PROGRAM_CACHE:
        _PROGRAM_CACHE[sched] = build_program(sched)
    nc = _PROGRAM_CACHE[sched]

    res = run_bass_kernel_spmd(nc, per_core, core_ids=list(range(N_CORES)))
    args = np.concatenate(
        [np.asarray(res.results[c]["out"]).reshape(BLKS * P, ROLE_DIM)
         for c in range(N_CORES)], axis=0)[slot].astype(np.float32)
    out = np.empty((N_TRIG, OUT_W), np.float32)
    out[:, 0:ENT_DIM] = np.asarray(inputs["ent_embeds"],
                                   np.float32)[np.asarray(
                                       inputs["trig_ent_id"], np.int64)]
    out[:, ENT_DIM:] = args
    return out


# revision 38
# speedup vs baseline: 1.0122x; 1.0099x over previous
"""Trainium2 Bass kernel for nn_EVModel (gnn_message_passing).

Strategy (8 NeuronCores, SPMD, no collectives):
  - Host: shard the 250k argument edges by owning trigger across the 8
    cores (49 blocks of 128 triggers per core).  Per the sharding hint the
    host gathers the per-edge rows ([rel | rtype | ent], 576 cols) into
    dense bf16 slabs, one [128, T_b*576 + T_b] tile per trigger block
    (x-rows + one-hot codes), with edges grouped by in/out side
    (pure-side tiles).  Triggers are packed into blocks by a degree-aware
    greedy so most blocks need only 5 edge-tiles (3 in + 2 out or 2+3);
    the per-block tile schedule is shared by all cores (SPMD).
  - Device, per block (dense HWDGE loads; no indirect DMA):
      * one DMA load of the block slab (sync queue)
      * per 128-edge tile: DVE builds a one-hot [128e,128t] from the
        codes; PE matmuls with the x-chunks as stationary operands
        accumulate A^T (the per-side segment-sum, transposed) directly in
        PSUM: A^T[c] += X_c^T @ OH  -- no transposes needed.
      * 9 accumulating PE matmuls A^T-chunk @ W-chunk -> args [128t,256]
        (the two 64-row tail chunks of the two sides share one matmul via
        an output-partition-offset segment-sum)
      * one store of args (scalar queue, also HWDGE)
  - Host: gather per-trigger args rows from the 8 slabs, concatenate the
    exact f32 trigger-entity rows (pure pass-through data movement).

Math identity: segsum(x*m) @ W_in + segsum(x*(1-m)) @ W_out, evaluated as
(OH_side^T @ X)^T accumulated transposed, then one 1152-deep matmul with
[W_in ; W_out].  bf16 storage/compute, f32 PSUM accumulation.
"""

import os
import sys

for _p in ("/opt/trn_rl_repo", "/root/.axon_site/_ro/trn_rl_repo"):
    if os.path.isdir(_p) and _p not in sys.path:
        sys.path.insert(0, _p)

import numpy as np
import ml_dtypes

BF16 = ml_dtypes.bfloat16

# ---------------------------------------------------------------- constants
N_ENT, N_REL, N_TRIG, N_ARGS = 100000, 250000, 50000, 250000
ENT_DIM, REL_R, RTYPE_DIM, ROLE_DIM, REL_SIZE = 288, 256, 32, 256, 200
ARG_DIM = REL_R + RTYPE_DIM + ENT_DIM          # 576
OUT_W = ENT_DIM + ROLE_DIM                     # 544
N_CORES = 8
P = 128                                        # partitions / trigger block
BLKS = 49                                      # trigger blocks per core
NBINS = N_CORES * BLKS                         # 392 trigger bins
CW = [128, 128, 128, 128, 64]                  # x-col chunk widths (576)
NWCH = 9                                       # W chunks (64-tails merged)


# ---------------------------------------------------------------- device code
def build_body(nc, tc, aps, sched):
    """Per-core Tile program.  aps: x [P, WTOT] bf16 (all block slabs,
    side by side), w [P, 9*256] bf16, iota [P,P] bf16,
    out [BLKS,P,256] bf16.  sched: list of (tin_b, tout_b) per block.

    Block slab layout per partition: [ x rows of tile 0..T_b-1 | codes ].
    Tiles 0..tin-1 hold in-edges, the rest out-edges; code 128 = pad.
    """
    import concourse.mybir as mybir
    f32, bf16 = mybir.dt.float32, mybir.dt.bfloat16
    eq = mybir.AluOpType.is_equal
    X, W, IOTA, OUT = aps["x"], aps["w"], aps["iota"], aps["out"]
    CODES = aps["codes"]
    t_tot = sum(t[0] + t[1] for t in sched)

    with (
        tc.tile_pool(name="const", bufs=1) as cpool,
        tc.tile_pool(name="xp", bufs=6) as xpool,
        tc.tile_pool(name="ohp", bufs=4) as ohpool,
        tc.tile_pool(name="atp", bufs=2) as atpool,
        tc.tile_pool(name="ap", bufs=3) as apool,
        tc.tile_pool(name="psa", bufs=2, space="PSUM") as psa,
        tc.tile_pool(name="psg", bufs=2, space="PSUM") as psg,
    ):
        wsb = cpool.tile([P, NWCH * ROLE_DIM], bf16, name="wsb")
        nc.sync.dma_start(out=wsb[:], in_=W[:])
        iota_sb = cpool.tile([P, P], bf16, name="iota_sb")
        nc.sync.dma_start(out=iota_sb[:], in_=IOTA[:])
        codes_sb = cpool.tile([P, t_tot], f32, name="codes_sb")
        nc.sync.dma_start(out=codes_sb[:], in_=CODES[:])

        def drain_w(b, atps):
            """Copies + W-stage for block b, emitted AFTER block b+1's
            segment-sum: the copies overlap the next segsum on PE and the
            ACT queue never parks on an unresolved wait."""
            # three wide PSUM->SBUF copies (fixed overhead dominates narrow
            # copies); atsb chunk order [in0-3 | out0-3 | merged tails]
            atsb = atpool.tile([P, NWCH * P], bf16, tag="atsb", name="atsb")
            nc.vector.tensor_copy(out=atsb[:, 0:4 * P], in_=atps[0][:, 0:4 * P])
            nc.scalar.copy(out=atsb[:, 4 * P:8 * P], in_=atps[1][:, 0:4 * P])
            nc.scalar.copy(out=atsb[:, 8 * P:9 * P], in_=atps[2][:, 0:P])
            # args = A_cat @ [W_in ; W_out]
            pargs = psg.tile([P, ROLE_DIM], f32, tag="pargs", name="pargs")
            for i in range(NWCH):
                nc.tensor.matmul(
                    out=pargs[:],
                    lhsT=atsb[:, i * P:(i + 1) * P],
                    rhs=wsb[:, i * ROLE_DIM:(i + 1) * ROLE_DIM],
                    start=(i == 0), stop=(i == NWCH - 1))
            return pargs

        def drain_store(b, pargs):
            """Args PSUM->SBUF + store for block b, emitted one block later
            still, so its wait (on the W matmuls) is already resolved when
            the ACT queue reaches it."""
            args_sb = apool.tile([P, ROLE_DIM], bf16, tag="args",
                                 name="args_sb")
            nc.scalar.copy(out=args_sb[:], in_=pargs[:])
            # store via SWDGE: the Pool engine is otherwise idle, so the
            # store's semaphore wait doesn't stall any compute queue
            nc.gpsimd.dma_start(out=OUT[b], in_=args_sb[:])

        off = 0
        ct = 0
        pending = None
        pending2 = None
        for b, (tin, tout) in enumerate(sched):
            T = tin + tout
            xw = T * ARG_DIM
            xall = xpool.tile([P, xw], bf16, tag="x")
            nc.sync.dma_start(out=xall[:], in_=X[:, off:off + xw])
            if b == 0:
                # PE clock warm-up: ~80 throwaway matmuls overlap the
                # prologue DMA wait so the real stream starts at full clock
                warm = psg.tile([P, ROLE_DIM], f32, tag="pargs", name="warm")
                for _ in range(56):
                    nc.tensor.matmul(out=warm[:, 0:P], lhsT=iota_sb[:],
                                     rhs=iota_sb[:], start=True, stop=True)

            # ---- segment-sum, transposed, A^T chunks packed into 3 PSUM
            # bank tiles.  HW: start=True marks pending-zero for the WHOLE
            # bank but only on the partitions written, so only the first
            # matmul touching a (bank, partition-range) uses start=True,
            # and all tenants of a bank cover the same partitions.  The two
            # 64-wide tail chunks share bank 2: in-tail on partitions 0:64,
            # out-tail on partitions 64:128 (via output partition offset).
            atps = [psa.tile([P, 4 * P], f32, tag=f"at{j}", name=f"at{j}")
                    for j in range(3)]

            def at_chunk(i, atps=atps):
                # i = side*5 + c -> (bank, region, partition range); the
                # two 64-wide tails pack into bank 2 cols 0:128 on disjoint
                # partition halves
                side, c = divmod(i, 5)
                if c < 4:
                    return atps[side][0:128, c * P:(c + 1) * P]
                if side == 0:
                    return atps[2][0:64, 0:P]
                return atps[2][64:128, 0:P]

            for k in range(T):
                oh = ohpool.tile([P, P], bf16, tag="oh")
                nc.vector.tensor_scalar(
                    out=oh[:], in0=iota_sb[:],
                    scalar1=codes_sb[:, ct + k:ct + k + 1],
                    scalar2=None, op0=eq)
                side = 0 if k < tin else 1
                first = (k == 0 or k == tin)
                for c, cw in enumerate(CW):
                    i = side * 5 + c
                    x0 = k * ARG_DIM + c * P
                    nc.tensor.matmul(out=at_chunk(i),
                                     lhsT=xall[:, x0:x0 + cw],
                                     rhs=oh[:],
                                     start=(first and c in (0, 4)),
                                     stop=(k == tin - 1 or k == T - 1),
                                     skip_group_check=True)

            if pending is not None:
                pb, patps = pending
                drain_store(pb, drain_w(pb, patps))
            pending = (b, atps)
            off += xw
            ct += T
        pb, patps = pending
        drain_store(pb, drain_w(pb, patps))


def build_program(sched):
    import concourse.bacc as bacc
    import concourse.mybir as mybir
    import concourse.tile as tile

    wtot = sum((tin + tout) * ARG_DIM for tin, tout in sched)
    t_tot = sum(tin + tout for tin, tout in sched)
    bf16 = mybir.dt.bfloat16
    f32 = mybir.dt.float32
    nc = bacc.Bacc("TRN2", target_bir_lowering=False, debug=False,
                   num_devices=N_CORES)
    aps = {
        "x": nc.dram_tensor("x", [P, wtot], bf16,
                            kind="ExternalInput").ap(),
        "w": nc.dram_tensor("w", [P, NWCH * ROLE_DIM], bf16,
                            kind="ExternalInput").ap(),
        "iota": nc.dram_tensor("iota", [P, P], bf16,
                               kind="ExternalInput").ap(),
        "codes": nc.dram_tensor("codes", [P, t_tot], f32,
                                kind="ExternalInput").ap(),
        "out": nc.dram_tensor("out", [BLKS, P, ROLE_DIM], bf16,
                              kind="ExternalOutput").ap(),
    }
    with tile.TileContext(nc) as tc:
        build_body(nc, tc, aps, sched)
    nc.compile()
    return nc


# ---------------------------------------------------------------- host prep
def pack_bins(din, dout):
    """Degree-aware greedy: pack triggers into 392 bins of 128 so each
    bin's per-side edge sums land just under multiples of 128.  Returns
    (bin_of_trigger, realized per-bin (in,out) sums)."""
    deg = np.stack([din, np.asarray(dout)], axis=1).astype(np.int64)
    # bucket triggers by (din, dout); virtual (0,0) triggers pad to 50176
    order = np.lexsort((deg[:, 1], deg[:, 0]))
    di, do = deg[order, 0], deg[order, 1]
    # bucket boundaries
    chg = np.nonzero((np.diff(di) != 0) | (np.diff(do) != 0))[0] + 1
    starts = np.concatenate([[0], chg])
    ends = np.concatenate([chg, [N_TRIG]])
    bi = di[starts].copy()
    bo = do[starts].copy()
    cnt = (ends - starts).astype(np.int64)
    pos = starts.copy()          # next unassigned trigger per bucket
    ndummy = NBINS * P - N_TRIG  # virtual zero-degree triggers
    if not (bi[0] == 0 and bo[0] == 0):
        bi = np.concatenate([[0], bi]); bo = np.concatenate([[0], bo])
        cnt = np.concatenate([[0], cnt]); pos = np.concatenate([[0], pos])
        starts = np.concatenate([[0], starts])
        ends = np.concatenate([[0], ends])
    cnt[0] += ndummy

    rem_in, rem_out = int(din.sum()), int(dout.sum())
    bin_of = np.empty(N_TRIG, np.int64)
    bin_fill = np.zeros(NBINS, np.int64)
    sums = np.zeros((NBINS, 2), np.int64)
    f = bi.astype(np.float64)
    g = bo.astype(np.float64)
    for q in range(NBINS):
        rb = NBINS - q
        # bin type: lean the side with more remaining edges
        ai = rem_in / rb
        ao = rem_out / rb
        ti = 128.0 * np.ceil(max(ai, 1.0) / 128.0)
        to = 128.0 * np.ceil(max(ao, 1.0) / 128.0)
        # prefer a 5-tile bin when the average allows
        if ti + to > 640 and ai + ao <= 636:
            if ai >= ao:
                ti, to = 384.0, 256.0
            else:
                ti, to = 256.0, 384.0
        si = so = 0.0
        for r in range(P, 0, -1):
            ni = (ti - si) / r
            no = (to - so) / r
            ok = cnt > 0
            # never overshoot the caps
            okc = ok & (f <= ti - si) & (g <= to - so)
            if not okc.any():
                okc = ok
            score = (f - ni) ** 2 + (g - no) ** 2
            score[~okc] = np.inf
            j = int(np.argmin(score))
            cnt[j] -= 1
            si += f[j]
            so += g[j]
            sums[q, 0] += bi[j]
            sums[q, 1] += bo[j]
            # assign a real trigger id if the bucket still has one
            # (bucket 0 also holds the virtual padding triggers)
            if pos[j] < ends[j]:
                bin_of[order[pos[j]]] = q
                pos[j] += 1
            bin_fill[q] += 1
        rem_in -= int(sums[q, 0])
        rem_out -= int(sums[q, 1])
    return bin_of, sums


def make_schedule(sums):
    """Group the 392 bins 8-per-block-index by tile class; returns
    (bin -> (core, blk) slot base order, sched list)."""
    ib = np.maximum(1, np.ceil(sums[:, 0] / P).astype(np.int64))
    ob = np.maximum(1, np.ceil(sums[:, 1] / P).astype(np.int64))
    order = np.lexsort((ob, ib))          # group equal classes together
    sched = []
    bin_slot = np.empty(NBINS, np.int64)  # bin -> global block slot
    for b in range(BLKS):
        grp = order[b * N_CORES:(b + 1) * N_CORES]
        sched.append((int(ib[grp].max()), int(ob[grp].max())))
        for c, q in enumerate(grp):
            bin_slot[q] = c * BLKS + b    # core c, block b
    return bin_slot, sched


def host_prep(inputs):
    rtype_ids = np.asarray(inputs["rtype_ids"], np.int64)
    arg_trig = np.asarray(inputs["arg_trig"], np.int64)
    arg_rel = np.asarray(inputs["arg_rel"], np.int64)
    arg_ent = np.asarray(inputs["arg_ent"], np.int64)
    arg_is_in = np.asarray(inputs["arg_is_in"], np.int64)

    rel_bf = np.asarray(inputs["rel_embeds"], np.float32).astype(BF16)
    rtt_bf = np.asarray(inputs["rtype_table"], np.float32).astype(BF16)
    ent_bf = np.asarray(inputs["ent_embeds"], np.float32).astype(BF16)

    din = np.bincount(arg_trig[arg_is_in == 1], minlength=N_TRIG)
    dout = np.bincount(arg_trig[arg_is_in == 0], minlength=N_TRIG)
    bin_of, sums = pack_bins(din, dout)
    bin_slot, sched = make_schedule(sums)

    # trigger -> (global block slot, position within block)
    blk_of_trig = bin_slot[bin_of]
    pos_order = np.lexsort((np.arange(N_TRIG), blk_of_trig))
    pos_of_trig = np.empty(N_TRIG, np.int64)
    pos_in_blk = np.arange(N_TRIG) - np.concatenate(
        [[0], np.cumsum(np.bincount(blk_of_trig, minlength=NBINS))])[
        blk_of_trig[pos_order]]
    pos_of_trig[pos_order] = pos_in_blk
    slot = blk_of_trig * P + pos_of_trig          # trigger -> output slot

    # per-block slab offsets
    tins = np.array([s[0] for s in sched], np.int64)
    touts = np.array([s[1] for s in sched], np.int64)
    ts = tins + touts
    xws = ts * ARG_DIM
    offs = np.concatenate([[0], np.cumsum(xws)])
    wtot = int(offs[-1])
    cts = np.concatenate([[0], np.cumsum(ts)])
    t_tot = int(cts[-1])

    # per-edge placement
    eblk = blk_of_trig[arg_trig]                  # global block slot (0..391)
    b_of = eblk % BLKS                            # block index within core
    core = eblk // BLKS
    epos = pos_of_trig[arg_trig]
    side = 1 - arg_is_in
    key = eblk * 2 + side
    order = np.argsort(key, kind="stable")
    ks = key[order]
    cntk = np.bincount(ks, minlength=2 * NBINS)
    grp_start = np.concatenate([[0], np.cumsum(cntk)])
    rank = np.arange(N_ARGS) - grp_start[ks]
    tin_e = tins[b_of[order]]
    ek = np.where(ks % 2 == 0, rank // P, tin_e + rank // P)
    ep = rank % P
    assert (np.where(ks % 2 == 0, rank, -1) < tin_e * P).all()
    assert (np.where(ks % 2 == 1, rank, -1) < touts[b_of[order]] * P).all()

    e = order
    X = np.zeros((N_CORES, P, wtot), BF16)
    flat = X.reshape(-1)
    base = (core[e] * P + ep) * wtot + offs[b_of[e]] + ek * ARG_DIM
    flat[base[:, None] + np.arange(REL_R)] = rel_bf[arg_rel[e]]
    flat[base[:, None] + (REL_R + np.arange(RTYPE_DIM))] = \
        rtt_bf[rtype_ids[arg_rel[e]]]
    flat[base[:, None] + (REL_R + RTYPE_DIM + np.arange(ENT_DIM))] = \
        ent_bf[arg_ent[e]]
    # codes (pad = 128), separate resident f32 table [cores, P, t_tot]
    codes = np.full((N_CORES, P, t_tot), np.float32(P), np.float32)
    codes.reshape(-1)[(core[e] * P + ep) * t_tot + cts[b_of[e]] + ek] = \
        epos[e].astype(np.float32)

    # W chunks [128, 9, 256] in atsb order [in0-3 | out0-3 | merged tails]
    wstack = np.concatenate([np.asarray(inputs["W_in"], np.float32),
                             np.asarray(inputs["W_out"], np.float32)], axis=0)
    wh = np.zeros((P, NWCH, ROLE_DIM), np.float32)
    for c in range(4):
        wh[:, c, :] = wstack[c * P:(c + 1) * P]
        wh[:, 4 + c, :] = wstack[ARG_DIM + c * P:ARG_DIM + (c + 1) * P]
    wh[0:64, 8, :] = wstack[512:576]
    wh[64:128, 8, :] = wstack[ARG_DIM + 512:ARG_DIM + 576]
    wh = wh.reshape(P, NWCH * ROLE_DIM).astype(BF16)

    iota = np.ascontiguousarray(
        np.broadcast_to(np.arange(P, dtype=np.float32), (P, P))).astype(BF16)

    per_core = [{"x": np.ascontiguousarray(X[c]), "w": wh, "iota": iota,
                 "codes": np.ascontiguousarray(codes[c])}
                for c in range(N_CORES)]
    return per_core, slot, tuple(sched)


_PROGRAM_CACHE = {}


def kernel(**inputs):
    from concourse.bass_utils import run_bass_kernel_spmd

    per_core, slot, sched = host_prep(inputs)
    if sched not in _PROGRAM_CACHE:
        _PROGRAM_CACHE[sched] = build_program(sched)
    nc = _PROGRAM_CACHE[sched]

    res = run_bass_kernel_spmd(nc, per_core, core_ids=list(range(N_CORES)))
    args = np.concatenate(
        [np.asarray(res.results[c]["out"]).reshape(BLKS * P, ROLE_DIM)
         for c in range(N_CORES)], axis=0)[slot].astype(np.float32)
    out = np.empty((N_TRIG, OUT_W), np.float32)
    out[:, 0:ENT_DIM] = np.asarray(inputs["ent_embeds"],
                                   np.float32)[np.asarray(
                                       inputs["trig_ent_id"], np.int64)]
    out[:, ENT_DIM:] = args
    return out


# revision 39
# speedup vs baseline: 1.0197x; 1.0074x over previous
"""Trainium2 Bass kernel for nn_EVModel (gnn_message_passing).

Strategy (8 NeuronCores, SPMD, no collectives):
  - Host: shard the 250k argument edges by owning trigger across the 8
    cores (49 blocks of 128 triggers per core).  Per the sharding hint the
    host gathers the per-edge rows ([rel | rtype | ent], 576 cols) into
    dense bf16 slabs, one [128, T_b*576 + T_b] tile per trigger block
    (x-rows + one-hot codes), with edges grouped by in/out side
    (pure-side tiles).  Triggers are packed into blocks by a degree-aware
    greedy so most blocks need only 5 edge-tiles (3 in + 2 out or 2+3);
    the per-block tile schedule is shared by all cores (SPMD).
  - Device, per block (dense HWDGE loads; no indirect DMA):
      * one DMA load of the block slab (sync queue)
      * per 128-edge tile: DVE builds a one-hot [128e,128t] from the
        codes; PE matmuls with the x-chunks as stationary operands
        accumulate A^T (the per-side segment-sum, transposed) directly in
        PSUM: A^T[c] += X_c^T @ OH  -- no transposes needed.
      * 9 accumulating PE matmuls A^T-chunk @ W-chunk -> args [128t,256]
        (the two 64-row tail chunks of the two sides share one matmul via
        an output-partition-offset segment-sum)
      * one store of args (scalar queue, also HWDGE)
  - Host: gather per-trigger args rows from the 8 slabs, concatenate the
    exact f32 trigger-entity rows (pure pass-through data movement).

Math identity: segsum(x*m) @ W_in + segsum(x*(1-m)) @ W_out, evaluated as
(OH_side^T @ X)^T accumulated transposed, then one 1152-deep matmul with
[W_in ; W_out].  bf16 storage/compute, f32 PSUM accumulation.
"""

import os
import sys

for _p in ("/opt/trn_rl_repo", "/root/.axon_site/_ro/trn_rl_repo"):
    if os.path.isdir(_p) and _p not in sys.path:
        sys.path.insert(0, _p)

import numpy as np
import ml_dtypes

BF16 = ml_dtypes.bfloat16

# ---------------------------------------------------------------- constants
N_ENT, N_REL, N_TRIG, N_ARGS = 100000, 250000, 50000, 250000
ENT_DIM, REL_R, RTYPE_DIM, ROLE_DIM, REL_SIZE = 288, 256, 32, 256, 200
ARG_DIM = REL_R + RTYPE_DIM + ENT_DIM          # 576
OUT_W = ENT_DIM + ROLE_DIM                     # 544
N_CORES = 8
P = 128                                        # partitions / trigger block
BLKS = 49                                      # trigger blocks per core
NBINS = N_CORES * BLKS                         # 392 trigger bins
CW = [128, 128, 128, 128, 64]                  # x-col chunk widths (576)
NWCH = 9                                       # W chunks (64-tails merged)


# ---------------------------------------------------------------- device code
def build_body(nc, tc, aps, sched):
    """Per-core Tile program.  aps: x [P, WTOT] bf16 (all block slabs,
    side by side), w [P, 9*256] bf16, iota [P,P] bf16,
    out [BLKS,P,256] bf16.  sched: list of (tin_b, tout_b) per block.

    Block slab layout per partition: [ x rows of tile 0..T_b-1 | codes ].
    Tiles 0..tin-1 hold in-edges, the rest out-edges; code 128 = pad.
    """
    import concourse.mybir as mybir
    f32, bf16 = mybir.dt.float32, mybir.dt.bfloat16
    eq = mybir.AluOpType.is_equal
    X, W, IOTA, OUT = aps["x"], aps["w"], aps["iota"], aps["out"]
    CODES = aps["codes"]
    t_tot = sum(t[0] + t[1] for t in sched)

    with (
        tc.tile_pool(name="const", bufs=1) as cpool,
        tc.tile_pool(name="xp", bufs=6) as xpool,
        tc.tile_pool(name="ohp", bufs=4) as ohpool,
        tc.tile_pool(name="atp", bufs=2) as atpool,
        tc.tile_pool(name="ap", bufs=3) as apool,
        tc.tile_pool(name="psa", bufs=2, space="PSUM") as psa,
        tc.tile_pool(name="psg", bufs=2, space="PSUM") as psg,
    ):
        wsb = cpool.tile([P, NWCH * ROLE_DIM], bf16, name="wsb")
        nc.sync.dma_start(out=wsb[:], in_=W[:])
        iota_sb = cpool.tile([P, P], bf16, name="iota_sb")
        nc.sync.dma_start(out=iota_sb[:], in_=IOTA[:])
        codes_sb = cpool.tile([P, t_tot], f32, name="codes_sb")
        nc.sync.dma_start(out=codes_sb[:], in_=CODES[:])

        def drain_w(b, atps):
            """Copies + W-stage for block b, emitted AFTER block b+1's
            segment-sum: the copies overlap the next segsum on PE and the
            ACT queue never parks on an unresolved wait."""
            # three wide PSUM->SBUF copies (fixed overhead dominates narrow
            # copies); atsb chunk order [in0-3 | out0-3 | merged tails]
            atsb = atpool.tile([P, NWCH * P], bf16, tag="atsb", name="atsb")
            nc.vector.tensor_copy(out=atsb[:, 0:4 * P], in_=atps[0][:, 0:4 * P])
            nc.scalar.copy(out=atsb[:, 4 * P:8 * P], in_=atps[1][:, 0:4 * P])
            nc.scalar.copy(out=atsb[:, 8 * P:9 * P], in_=atps[2][:, 0:P])
            # args = A_cat @ [W_in ; W_out]
            pargs = psg.tile([P, ROLE_DIM], f32, tag="pargs", name="pargs")
            for i in range(NWCH):
                nc.tensor.matmul(
                    out=pargs[:],
                    lhsT=atsb[:, i * P:(i + 1) * P],
                    rhs=wsb[:, i * ROLE_DIM:(i + 1) * ROLE_DIM],
                    start=(i == 0), stop=(i == NWCH - 1))
            return pargs

        def drain_store(b, pargs):
            """Args PSUM->SBUF + store for block b, emitted one block later
            still, so its wait (on the W matmuls) is already resolved when
            the ACT queue reaches it."""
            args_sb = apool.tile([P, ROLE_DIM], bf16, tag="args",
                                 name="args_sb")
            nc.scalar.copy(out=args_sb[:], in_=pargs[:])
            # store via SWDGE: the Pool engine is otherwise idle, so the
            # store's semaphore wait doesn't stall any compute queue
            nc.gpsimd.dma_start(out=OUT[b], in_=args_sb[:])

        off = 0
        ct = 0
        pending = None
        pending2 = None
        for b, (tin, tout) in enumerate(sched):
            T = tin + tout
            xw = T * ARG_DIM
            xall = xpool.tile([P, xw], bf16, tag="x")
            nc.sync.dma_start(out=xall[:], in_=X[:, off:off + xw])
            # ---- segment-sum, transposed, A^T chunks packed into 3 PSUM
            # bank tiles.  HW: start=True marks pending-zero for the WHOLE
            # bank but only on the partitions written, so only the first
            # matmul touching a (bank, partition-range) uses start=True,
            # and all tenants of a bank cover the same partitions.  The two
            # 64-wide tail chunks share bank 2: in-tail on partitions 0:64,
            # out-tail on partitions 64:128 (via output partition offset).
            atps = [psa.tile([P, 4 * P], f32, tag=f"at{j}", name=f"at{j}")
                    for j in range(3)]

            def at_chunk(i, atps=atps):
                # i = side*5 + c -> (bank, region, partition range); the
                # two 64-wide tails pack into bank 2 cols 0:128 on disjoint
                # partition halves
                side, c = divmod(i, 5)
                if c < 4:
                    return atps[side][0:128, c * P:(c + 1) * P]
                if side == 0:
                    return atps[2][0:64, 0:P]
                return atps[2][64:128, 0:P]

            for k in range(T):
                oh = ohpool.tile([P, P], bf16, tag="oh")
                nc.vector.tensor_scalar(
                    out=oh[:], in0=iota_sb[:],
                    scalar1=codes_sb[:, ct + k:ct + k + 1],
                    scalar2=None, op0=eq)
                side = 0 if k < tin else 1
                first = (k == 0 or k == tin)
                for c, cw in enumerate(CW):
                    i = side * 5 + c
                    x0 = k * ARG_DIM + c * P
                    nc.tensor.matmul(out=at_chunk(i),
                                     lhsT=xall[:, x0:x0 + cw],
                                     rhs=oh[:],
                                     start=(first and c in (0, 4)),
                                     stop=(k == tin - 1 or k == T - 1),
                                     skip_group_check=True)

            if pending is not None:
                pb, patps = pending
                drain_store(pb, drain_w(pb, patps))
            pending = (b, atps)
            off += xw
            ct += T
        pb, patps = pending
        drain_store(pb, drain_w(pb, patps))


def build_program(sched):
    import concourse.bacc as bacc
    import concourse.mybir as mybir
    import concourse.tile as tile

    wtot = sum((tin + tout) * ARG_DIM for tin, tout in sched)
    t_tot = sum(tin + tout for tin, tout in sched)
    bf16 = mybir.dt.bfloat16
    f32 = mybir.dt.float32
    nc = bacc.Bacc("TRN2", target_bir_lowering=False, debug=False,
                   num_devices=N_CORES)
    aps = {
        "x": nc.dram_tensor("x", [P, wtot], bf16,
                            kind="ExternalInput").ap(),
        "w": nc.dram_tensor("w", [P, NWCH * ROLE_DIM], bf16,
                            kind="ExternalInput").ap(),
        "iota": nc.dram_tensor("iota", [P, P], bf16,
                               kind="ExternalInput").ap(),
        "codes": nc.dram_tensor("codes", [P, t_tot], f32,
                                kind="ExternalInput").ap(),
        "out": nc.dram_tensor("out", [BLKS, P, ROLE_DIM], bf16,
                              kind="ExternalOutput").ap(),
    }
    with tile.TileContext(nc) as tc:
        build_body(nc, tc, aps, sched)
    nc.compile()
    return nc


# ---------------------------------------------------------------- host prep
def pack_bins(din, dout):
    """Degree-aware greedy: pack triggers into 392 bins of 128 so each
    bin's per-side edge sums land just under multiples of 128.  Returns
    (bin_of_trigger, realized per-bin (in,out) sums)."""
    deg = np.stack([din, np.asarray(dout)], axis=1).astype(np.int64)
    # bucket triggers by (din, dout); virtual (0,0) triggers pad to 50176
    order = np.lexsort((deg[:, 1], deg[:, 0]))
    di, do = deg[order, 0], deg[order, 1]
    # bucket boundaries
    chg = np.nonzero((np.diff(di) != 0) | (np.diff(do) != 0))[0] + 1
    starts = np.concatenate([[0], chg])
    ends = np.concatenate([chg, [N_TRIG]])
    bi = di[starts].copy()
    bo = do[starts].copy()
    cnt = (ends - starts).astype(np.int64)
    pos = starts.copy()          # next unassigned trigger per bucket
    ndummy = NBINS * P - N_TRIG  # virtual zero-degree triggers
    if not (bi[0] == 0 and bo[0] == 0):
        bi = np.concatenate([[0], bi]); bo = np.concatenate([[0], bo])
        cnt = np.concatenate([[0], cnt]); pos = np.concatenate([[0], pos])
        starts = np.concatenate([[0], starts])
        ends = np.concatenate([[0], ends])
    cnt[0] += ndummy

    rem_in, rem_out = int(din.sum()), int(dout.sum())
    bin_of = np.empty(N_TRIG, np.int64)
    bin_fill = np.zeros(NBINS, np.int64)
    sums = np.zeros((NBINS, 2), np.int64)
    f = bi.astype(np.float64)
    g = bo.astype(np.float64)
    for q in range(NBINS):
        rb = NBINS - q
        # bin type: lean the side with more remaining edges
        ai = rem_in / rb
        ao = rem_out / rb
        ti = 128.0 * np.ceil(max(ai, 1.0) / 128.0)
        to = 128.0 * np.ceil(max(ao, 1.0) / 128.0)
        # prefer a 5-tile bin when the average allows
        if ti + to > 640 and ai + ao <= 636:
            if ai >= ao:
                ti, to = 384.0, 256.0
            else:
                ti, to = 256.0, 384.0
        si = so = 0.0
        for r in range(P, 0, -1):
            ni = (ti - si) / r
            no = (to - so) / r
            ok = cnt > 0
            # never overshoot the caps
            okc = ok & (f <= ti - si) & (g <= to - so)
            if not okc.any():
                okc = ok
            score = (f - ni) ** 2 + (g - no) ** 2
            score[~okc] = np.inf
            j = int(np.argmin(score))
            cnt[j] -= 1
            si += f[j]
            so += g[j]
            sums[q, 0] += bi[j]
            sums[q, 1] += bo[j]
            # assign a real trigger id if the bucket still has one
            # (bucket 0 also holds the virtual padding triggers)
            if pos[j] < ends[j]:
                bin_of[order[pos[j]]] = q
                pos[j] += 1
            bin_fill[q] += 1
        rem_in -= int(sums[q, 0])
        rem_out -= int(sums[q, 1])
    return bin_of, sums


def make_schedule(sums):
    """Group the 392 bins 8-per-block-index by tile class; returns
    (bin -> (core, blk) slot base order, sched list)."""
    ib = np.maximum(1, np.ceil(sums[:, 0] / P).astype(np.int64))
    ob = np.maximum(1, np.ceil(sums[:, 1] / P).astype(np.int64))
    order = np.lexsort((ob, ib))          # group equal classes together
    sched = []
    bin_slot = np.empty(NBINS, np.int64)  # bin -> global block slot
    for b in range(BLKS):
        grp = order[b * N_CORES:(b + 1) * N_CORES]
        sched.append((int(ib[grp].max()), int(ob[grp].max())))
        for c, q in enumerate(grp):
            bin_slot[q] = c * BLKS + b    # core c, block b
    return bin_slot, sched


def host_prep(inputs):
    rtype_ids = np.asarray(inputs["rtype_ids"], np.int64)
    arg_trig = np.asarray(inputs["arg_trig"], np.int64)
    arg_rel = np.asarray(inputs["arg_rel"], np.int64)
    arg_ent = np.asarray(inputs["arg_ent"], np.int64)
    arg_is_in = np.asarray(inputs["arg_is_in"], np.int64)

    rel_bf = np.asarray(inputs["rel_embeds"], np.float32).astype(BF16)
    rtt_bf = np.asarray(inputs["rtype_table"], np.float32).astype(BF16)
    ent_bf = np.asarray(inputs["ent_embeds"], np.float32).astype(BF16)

    din = np.bincount(arg_trig[arg_is_in == 1], minlength=N_TRIG)
    dout = np.bincount(arg_trig[arg_is_in == 0], minlength=N_TRIG)
    bin_of, sums = pack_bins(din, dout)
    bin_slot, sched = make_schedule(sums)

    # trigger -> (global block slot, position within block)
    blk_of_trig = bin_slot[bin_of]
    pos_order = np.lexsort((np.arange(N_TRIG), blk_of_trig))
    pos_of_trig = np.empty(N_TRIG, np.int64)
    pos_in_blk = np.arange(N_TRIG) - np.concatenate(
        [[0], np.cumsum(np.bincount(blk_of_trig, minlength=NBINS))])[
        blk_of_trig[pos_order]]
    pos_of_trig[pos_order] = pos_in_blk
    slot = blk_of_trig * P + pos_of_trig          # trigger -> output slot

    # per-block slab offsets
    tins = np.array([s[0] for s in sched], np.int64)
    touts = np.array([s[1] for s in sched], np.int64)
    ts = tins + touts
    xws = ts * ARG_DIM
    offs = np.concatenate([[0], np.cumsum(xws)])
    wtot = int(offs[-1])
    cts = np.concatenate([[0], np.cumsum(ts)])
    t_tot = int(cts[-1])

    # per-edge placement
    eblk = blk_of_trig[arg_trig]                  # global block slot (0..391)
    b_of = eblk % BLKS                            # block index within core
    core = eblk // BLKS
    epos = pos_of_trig[arg_trig]
    side = 1 - arg_is_in
    key = eblk * 2 + side
    order = np.argsort(key, kind="stable")
    ks = key[order]
    cntk = np.bincount(ks, minlength=2 * NBINS)
    grp_start = np.concatenate([[0], np.cumsum(cntk)])
    rank = np.arange(N_ARGS) - grp_start[ks]
    tin_e = tins[b_of[order]]
    ek = np.where(ks % 2 == 0, rank // P, tin_e + rank // P)
    ep = rank % P
    assert (np.where(ks % 2 == 0, rank, -1) < tin_e * P).all()
    assert (np.where(ks % 2 == 1, rank, -1) < touts[b_of[order]] * P).all()

    e = order
    X = np.zeros((N_CORES, P, wtot), BF16)
    flat = X.reshape(-1)
    base = (core[e] * P + ep) * wtot + offs[b_of[e]] + ek * ARG_DIM
    flat[base[:, None] + np.arange(REL_R)] = rel_bf[arg_rel[e]]
    flat[base[:, None] + (REL_R + np.arange(RTYPE_DIM))] = \
        rtt_bf[rtype_ids[arg_rel[e]]]
    flat[base[:, None] + (REL_R + RTYPE_DIM + np.arange(ENT_DIM))] = \
        ent_bf[arg_ent[e]]
    # codes (pad = 128), separate resident f32 table [cores, P, t_tot]
    codes = np.full((N_CORES, P, t_tot), np.float32(P), np.float32)
    codes.reshape(-1)[(core[e] * P + ep) * t_tot + cts[b_of[e]] + ek] = \
        epos[e].astype(np.float32)

    # W chunks [128, 9, 256] in atsb order [in0-3 | out0-3 | merged tails]
    wstack = np.concatenate([np.asarray(inputs["W_in"], np.float32),
                             np.asarray(inputs["W_out"], np.float32)], axis=0)
    wh = np.zeros((P, NWCH, ROLE_DIM), np.float32)
    for c in range(4):
        wh[:, c, :] = wstack[c * P:(c + 1) * P]
        wh[:, 4 + c, :] = wstack[ARG_DIM + c * P:ARG_DIM + (c + 1) * P]
    wh[0:64, 8, :] = wstack[512:576]
    wh[64:128, 8, :] = wstack[ARG_DIM + 512:ARG_DIM + 576]
    wh = wh.reshape(P, NWCH * ROLE_DIM).astype(BF16)

    iota = np.ascontiguousarray(
        np.broadcast_to(np.arange(P, dtype=np.float32), (P, P))).astype(BF16)

    per_core = [{"x": np.ascontiguousarray(X[c]), "w": wh, "iota": iota,
                 "codes": np.ascontiguousarray(codes[c])}
                for c in range(N_CORES)]
    return per_core, slot, tuple(sched)


_PROGRAM_CACHE = {}


def kernel(**inputs):
    from concourse.bass_utils import run_bass_kernel_spmd

    per_core, slot, sched = host_prep(inputs)
    if sched not in _PROGRAM_CACHE:
        _PROGRAM_CACHE[sched] = build_program(sched)
    nc = _PROGRAM_CACHE[sched]

    res = run_bass_kernel_spmd(nc, per_core, core_ids=list(range(N_CORES)))
    args = np.concatenate(
        [np.asarray(res.results[c]["out"]).reshape(BLKS * P, ROLE_DIM)
         for c in range(N_CORES)], axis=0)[slot].astype(np.float32)
    out = np.empty((N_TRIG, OUT_W), np.float32)
    out[:, 0:ENT_DIM] = np.asarray(inputs["ent_embeds"],
                                   np.float32)[np.asarray(
                                       inputs["trig_ent_id"], np.int64)]
    out[:, ENT_DIM:] = args
    return out
